# revision 15
# baseline (speedup 1.0000x reference)
"""Butterfly sparse-attention MLP kernel for 8 Trainium2 NeuronCores.

Computation (from the reference):
    attn = (w1.T @ w2.T) * sparse_mask          # [4096 s, 4096 t]
    y    = gelu(x @ attn + b2)                  # [8, 768, 4096]

sparse_mask is banded: mask[s, t] == 0 whenever |s - t| > 133.  Each core
owns a 512-wide t-block and needs only a 784-wide s-window around it
(133 before + 512 + 139 after, the last 6 rows zero padding).  Per
t-subtile of 128, only 394 s-rows are in band, so phase B contracts over
3 full 128-chunks plus a 16-row chunk, and phase A computes only the
exact in-band t-range of each attn chunk (the rest of each attn SBUF
tile is memset to zero).

Sharding: tensor-parallel over t (8 blocks of 512).  All per-core variation
is in the input data (windows are zero-padded at the edges; mask zeros make
padded contributions exactly zero), so one SPMD BIR serves all 8 cores.

The kernel is DMA-bound at the ~410 GB/s per-core fabric cap, so every
stream is host-packed so DMA rows are 4 KB (or large) multiples: w1 is one
[128, 32*784] tensor pulled in 3 slices, w2 rows are 4 KB, x rows are 4 KB
slices of 12 KB rows, y stores are [128, 2048] tiles.  Weights get the
bandwidth first; x trickles behind them (paced by deps on the weight DMAs)
so phase B can start the moment attn is ready.  The scalar engine does no
DMA during phase B so gelu activations never stall the y stream.
"""

import numpy as np

B, T, D = 8, 768, 4096
N = B * T            # 6144 rows of x
NCORES = 8
TB = 512             # t-columns per core
P = 128
M0 = 133             # s-window starts this far before the t-block
SW = 784             # s-window width (133 + 512 + 133 + 6 pad)
NCH = 7              # s-chunks: 6 full + one 16-row chunk
LASTROWS = 16        # rows in chunk 6 (10 in-band + 6 pad)
DCH = D // P         # 32 d-chunks (contraction of phase A)
NQ = TB // P         # 4 t-subtiles per core
GN = 2048            # n-group width in phase B
NG = N // GN         # 3 n-groups
BANDCH = 4           # s-chunks feeding one t-subtile (covers +-133 band)
W1SPLIT = (2, 5, 5, 5, 5, 5, 5)  # w1 d-chunks per DMA slice (small first
# piece so the PE starts early; 5-chunk rows are 7840 B = 1.91 x 4 KB
# packets, 95.7% packet efficiency)

# Exact in-band t-range [lo_j, hi_j) of attn chunk j (window coords with
# M0 = 133: rows of chunk j are s - t0 + 133 in [128j, 128j+rows)).
BAND_LO = (0, 0, 0, 118, 246, 374, 502)
BAND_HI = (128, 256, 384, 512, 512, 512, 512)
BAND_W = tuple(h - l for l, h in zip(BAND_LO, BAND_HI))
MOFF = tuple(sum(BAND_W[:j]) for j in range(NCH))  # mask col offsets
MW = sum(BAND_W)  # 1576

_NC = None


def _build_module():
    from concourse import bacc, bass, mybir, tile
    from concourse.tile_rust import add_dep_helper

    f32 = mybir.dt.float32
    f16 = mybir.dt.float16
    PSUM = bass.MemorySpace.PSUM

    nc = bacc.Bacc("TRN2", target_bir_lowering=False, debug=False)
    xT_d = nc.declare_dram_parameter("xT_s", [NCH - 1, P, N], f16, isOutput=False)
    x6_d = nc.declare_dram_parameter("x6_s", [LASTROWS, N], f16, isOutput=False)
    w1_d = nc.declare_dram_parameter("w1_s", [P, DCH * SW], f16, isOutput=False)
    w2T_d = nc.declare_dram_parameter("w2T_s", [DCH // 4, P, 4 * TB], f16,
                                      isOutput=False)
    mask_d = nc.declare_dram_parameter("mask_s", [P, MW], f16, isOutput=False)
    b2_d = nc.declare_dram_parameter("b2c_s", [P, NQ], f32, isOutput=False)
    yT_d = nc.declare_dram_parameter("yT_s", [TB, N], f16, isOutput=True)

    with tile.TileContext(nc) as tc:
        with (
            tc.tile_pool(name="const", bufs=1) as cpool,
            tc.tile_pool(name="attn", bufs=1) as apool,
            tc.tile_pool(name="xp", bufs=NG * NCH) as xp,
            tc.tile_pool(name="yp", bufs=3) as yp,
        ):
            b2_t = cpool.tile([P, NQ], f32)
            m_t = cpool.tile([P, MW], f16)

            # attn tiles are full 512 wide; zero them so phase B's 128-wide
            # stationary reads see zeros outside the exact band.
            attn_sb = []
            for j in range(NCH):
                a_t = apool.tile([P, TB], f16, name=f"attn_sb{j}")
                nc.vector.memset(a_t[:], 0.0)
                attn_sb.append(a_t)

            w_insts = []

            # ---- Phase A: attn[s, t] = (w1.T @ w2T) * mask on the band ----
            with (
                tc.tile_pool(name="w1p", bufs=1) as w1p,
                tc.tile_pool(name="w2p", bufs=DCH // 4) as w2p,
                tc.tile_pool(name="psA", bufs=1, space=PSUM) as psA,
            ):
                w1_t = w1p.tile([P, DCH * SW], f16)
                w2_ts = []
                # Interleave w1 slices with w2 chunk loads so phase A's
                # k-consumption order matches arrival order.  The first w2
                # tile is filled by two DMAs so the k=0 matmul only waits
                # for its first 512 columns.
                splits = [int(v) * SW for v in np.cumsum((0,) + W1SPLIT)]
                w2i = 0

                def load_w2(upto):
                    nonlocal w2i
                    while w2i < upto:
                        w2_t = w2p.tile([P, 4 * TB], f16, name=f"w2_{w2i}",
                                        tag="w2")
                        if w2i == 0:
                            w_insts.append(nc.sync.dma_start(
                                w2_t[:, 0:TB], w2T_d[0, :, 0:TB]))
                            w_insts.append(nc.sync.dma_start(
                                w2_t[:, TB:4 * TB], w2T_d[0, :, TB:4 * TB]))
                        else:
                            w_insts.append(
                                nc.sync.dma_start(w2_t[:], w2T_d[w2i]))
                        w2_ts.append(w2_t)
                        w2i += 1

                load_w2(1)
                for pi in range(len(W1SPLIT)):
                    w_insts.append(nc.gpsimd.dma_start(
                        w1_t[:, splits[pi]:splits[pi + 1]],
                        w1_d[:, splits[pi]:splits[pi + 1]]))
                    # w2 chunks needed for this piece's k-range
                    load_w2(min(DCH, sum(W1SPLIT[:pi + 1]) + 4) // 4)
                load_w2(DCH // 4)
                # b2 is 128 tiny descriptors; needed only at the first
                # activation, so it goes last on the w2 queue.
                nc.sync.dma_start(b2_t[:], b2_d[:])

                attn_ps = [
                    psA.tile([P, BAND_W[j]], f32, name=f"attn_ps{j}")
                    for j in range(NCH)
                ]
                for k in range(DCH):
                    w1row = w1_t[:, k * SW:(k + 1) * SW]
                    w2row = w2_ts[k // 4][:, (k % 4) * TB:(k % 4 + 1) * TB]
                    # last k in j-ascending order so the mask-muls (and
                    # phase B's first subtile) unblock in consumption order
                    jorder = range(NCH) if k == DCH - 1 else (3, 2, 4, 1, 5, 0, 6)
                    for j in jorder:
                        scols = LASTROWS if j == 6 else P
                        nc.tensor.matmul(
                            attn_ps[j][0:scols, 0:BAND_W[j]],
                            w1row[:, j * P:j * P + scols],
                            w2row[:, BAND_LO[j]:BAND_HI[j]],
                            start=(k == 0),
                            stop=(k == DCH - 1),
                        )
                for j in range(NCH):
                    rows = LASTROWS if j == 6 else P
                    nc.vector.tensor_mul(
                        attn_sb[j][0:rows, BAND_LO[j]:BAND_HI[j]],
                        attn_ps[j][0:rows, 0:BAND_W[j]],
                        m_t[0:rows, MOFF[j]:MOFF[j] + BAND_W[j]],
                    )

            # x prefetch: paced behind the weight stream so weights keep
            # most of the bandwidth until phase A's inputs are in.
            # w_insts order: [w2_0a, w2_0b, w1p0, w1p1, w2_1, w1p2, w2_2,
            #   w2_3, w1p3, w2_4, w1p4, w2_5, w1p5, w2_6, w1p6, w2_7].
            # The 4 x chunks phase B touches first ride the scalar HWDGE
            # queue, paced by engine-stall gates on the weight DMAs.  All
            # other x chunks go on the gpsimd ring BEHIND the w1 pieces —
            # ring FIFO sequences their bytes after w1's without stalling
            # any engine.
            x_t = [[None] * NCH for _ in range(NG)]
            gates = {(0, 0): 4, (0, 1): 6, (0, 2): 8, (0, 3): 10}
            for g in range(NG):
                for j in range(NCH):
                    if j == 6:
                        xt = xp.tile([LASTROWS, GN], f16, name="x6_t",
                                     tag="x_t")
                        src = x6_d[:, g * GN:(g + 1) * GN]
                    else:
                        xt = xp.tile([P, GN], f16, name="x_t", tag="x_t")
                        src = xT_d[j, :, g * GN:(g + 1) * GN]
                    gate = gates.get((g, j))
                    if gate is not None:
                        xi = nc.scalar.dma_start(xt[:], src)
                        add_dep_helper(
                            xi.ins, w_insts[gate].ins,
                            sync=True, reason="pace x behind weights",
                        )
                    else:
                        nc.gpsimd.dma_start(xt[:], src)
                    x_t[g][j] = xt
                    if (g, j) == (0, 3):
                        # mask is needed right at phase A's end
                        mi = nc.scalar.dma_start(m_t[:], mask_d[:])
                        add_dep_helper(
                            mi.ins, w_insts[12].ins,
                            sync=True, reason="mask after most weights",
                        )

            # ---- Phase B: yT[t, n] = gelu(attn.T @ xT + b2) on the band ----
            with tc.tile_pool(name="psB", bufs=4, space=PSUM) as psB:
                for g in range(NG):
                    for q in range(NQ):
                        y_sb = yp.tile([P, GN], f16, name="y_sb", tag="y_sb")
                        for h in range(2):
                            y_ps = psB.tile([P, GN // 2], f32, name="y_ps",
                                            tag="y_ps")
                            for hh in range(2):
                                osl = slice(hh * 512, (hh + 1) * 512)
                                nsl = slice((2 * h + hh) * 512,
                                            (2 * h + hh + 1) * 512)
                                for c in range(BANDCH):
                                    j = q + c
                                    rows = LASTROWS if j == 6 else P
                                    nc.tensor.matmul(
                                        y_ps[:, osl],
                                        attn_sb[j][0:rows,
                                                   q * P:(q + 1) * P],
                                        x_t[g][j][0:rows, nsl],
                                        start=(c == 0),
                                        stop=(c == BANDCH - 1),
                                    )
                            nc.scalar.activation(
                                y_sb[:, h * (GN // 2):(h + 1) * (GN // 2)],
                                y_ps[:],
                                mybir.ActivationFunctionType.Gelu,
                                bias=b2_t[:, q:q + 1],
                                scale=1.0,
                            )
                            # store per activation so the final store chain
                            # after the last matmul is as short as possible
                            nc.sync.dma_start(
                                yT_d[q * P:(q + 1) * P,
                                     g * GN + h * (GN // 2):
                                     g * GN + (h + 1) * (GN // 2)],
                                y_sb[:, h * (GN // 2):(h + 1) * (GN // 2)],
                            )

    nc.compile()
    nc.finalize()
    return nc


def _get_nc():
    global _NC
    if _NC is None:
        _NC = _build_module()
    return _NC


def prepare_in_maps(x, w1, w2, b2, sparse_mask):
    x = np.asarray(x, dtype=np.float32)
    w1 = np.asarray(w1, dtype=np.float32)
    w2 = np.asarray(w2, dtype=np.float32)
    b2 = np.asarray(b2, dtype=np.float32)
    sparse_mask = np.asarray(sparse_mask, dtype=np.float32)

    xT = np.ascontiguousarray(x.reshape(N, D).T.astype(np.float16))   # [s, n]
    w2T = np.ascontiguousarray(w2.T.astype(np.float16))               # [d, t]

    # Zero-pad the s axis (133 left, 139 right) so every core's window is
    # a plain slice; mask zeros make the padded rows contribute nothing.
    PADL, PADR = M0, SW - TB - M0
    xT_pad = np.zeros((D + PADL + PADR, N), dtype=np.float16)
    xT_pad[PADL:PADL + D] = xT
    w1_pad = np.zeros((D, D + PADL + PADR), dtype=np.float16)
    w1_pad[:, PADL:PADL + D] = w1.astype(np.float16)
    mask_pad = np.zeros((D + PADL + PADR, D), dtype=np.float16)
    mask_pad[PADL:PADL + D] = sparse_mask.astype(np.float16)

    in_maps = []
    for i in range(NCORES):
        s0 = i * TB           # window start in padded coords
        t0 = i * TB
        win = xT_pad[s0:s0 + SW]                          # [SW, N]
        # w1 window packed so each partition's 32 chunks are contiguous:
        # w1_s[p, k*SW + c] = w1_pad[128k + p, s0 + c]
        w1win = w1_pad[:, s0:s0 + SW]                     # [D, SW]
        w1_s = np.ascontiguousarray(
            w1win.reshape(DCH, P, SW).transpose(1, 0, 2).reshape(P, DCH * SW))
        w2win = w2T[:, t0:t0 + TB]                        # [D, TB]
        w2_s = (w2win.reshape(DCH // 4, 4, P, TB)
                .transpose(0, 2, 1, 3)
                .reshape(DCH // 4, P, 4 * TB))
        # mask packed per chunk at its exact band: [128, MW]
        mask_s = np.zeros((P, MW), dtype=np.float16)
        mwin = mask_pad[s0:s0 + SW, t0:t0 + TB]           # [SW, TB]
        for j in range(NCH):
            rows = LASTROWS if j == NCH - 1 else P
            mask_s[0:rows, MOFF[j]:MOFF[j] + BAND_W[j]] = (
                mwin[j * P:j * P + rows, BAND_LO[j]:BAND_HI[j]])
        in_maps.append({
            "xT_s": np.ascontiguousarray(
                win[:(NCH - 1) * P].reshape(NCH - 1, P, N)),
            "x6_s": np.ascontiguousarray(win[(NCH - 1) * P:]),
            "w1_s": w1_s,
            "w2T_s": np.ascontiguousarray(w2_s),
            "mask_s": mask_s,
            "b2c_s": np.ascontiguousarray(b2[t0:t0 + TB].reshape(NQ, P).T),
        })
    return in_maps


def assemble(results):
    out = np.empty((N, D), dtype=np.float32)
    for i in range(NCORES):
        out[:, i * TB:(i + 1) * TB] = results[i]["yT_s"].T.astype(np.float32)
    return out.reshape(B, T, D)


def _band_ok(sparse_mask):
    """The Bass kernel only computes attn on the exact per-chunk bands;
    verify every mask nonzero falls inside that region."""
    s_idx, t_idx = np.nonzero(np.asarray(sparse_mask) != 0)
    if len(s_idx) == 0:
        return True
    w0 = (t_idx // TB) * TB - M0              # per-core s-window start
    r = s_idx - w0                            # row in window coords
    if not np.all((r >= 0) & (r < SW - 6)):
        return False
    j = r // P
    tp = t_idx % TB
    lo = np.asarray(BAND_LO)[j]
    hi = np.asarray(BAND_HI)[j]
    return bool(np.all((tp >= lo) & (tp < hi)))


def _reference_fallback(x, w1, w2, b2, sparse_mask):
    import jax
    import jax.numpy as jnp

    cpu = jax.devices("cpu")[0]
    with jax.default_device(cpu):
        attn = jnp.einsum("ds,td->st", jnp.asarray(w1), jnp.asarray(w2))
        attn = attn * jnp.asarray(sparse_mask)
        y = jnp.einsum("bds,st->bdt", jnp.asarray(x), attn) + jnp.asarray(b2)
        return np.asarray(jax.nn.gelu(y, approximate=False), dtype=np.float32)


def kernel(x, w1, w2, b2, sparse_mask):
    import time

    from concourse.bass_utils import run_bass_kernel_spmd

    if (np.shape(x) != (B, T, D) or np.shape(w1) != (D, D)
            or np.shape(w2) != (D, D) or np.shape(b2) != (D,)
            or np.shape(sparse_mask) != (D, D) or not _band_ok(sparse_mask)):
        return _reference_fallback(x, w1, w2, b2, sparse_mask)

    in_maps = prepare_in_maps(x, w1, w2, b2, sparse_mask)
    nc = _get_nc()
    last_err = None
    for attempt in range(3):
        try:
            res = run_bass_kernel_spmd(nc, in_maps, list(range(NCORES)))
            return assemble(res.results)
        except Exception as e:  # transient NRT/device errors: retry
            last_err = e
            time.sleep(2.0 * (attempt + 1))
    raise last_err
